# revision 20
# baseline (speedup 1.0000x reference)
"""Bicubic 4x upsample (Keys a=-0.75) on 8 Trainium2 NeuronCores.

Strategy
--------
Data parallel over the batch: core i handles images [2i, 2i+1] (6
image-channels of 256x256 each), no cross-core communication.

Per image-channel the separable bicubic upsample is expressed as two
banded matmuls on the TensorEngine with NO transposes:

  pass A:  t1t[wi, ho] = sum_hi xp[hi, wi] * Ut[hi, ho]      (vertical)
  pass B:  out[ho, wo] = sum_wi t1t[wi, ho] * Ut[wi, wo]     (horizontal)

where xp is the edge-padded [259, 259] input (as stored: partitions =
rows = hi) and Ut [259, 1024] is the transposed upsample matrix
Ut[i+j, 4i+d] = K[d, j].  Both passes use matmul(out, lhsT, rhs) =
lhsT.T @ rhs: pass A takes lhsT = xp (already [hi, wi]), pass B takes
lhsT = t1t (already [wi, ho]).

Banding: an output column chunk [512n, 512n+512) needs contraction
rows [128n, 128n+131).  We supply it as one K=128 matmul over the full
512 columns plus one K=3 accumulating matmul over ONLY the last 12
columns -- Ut's 4-tap band means rows [128(n+1), 128(n+1)+3) touch
just columns [512(n+1)-12, 512(n+1)).  (The previous version ran the
K=3 correction over all 512 columns; PE matmul cost is driven by the
streamed column count, so that doubled TensorE time for no reason.)

Everything is float16 end to end: the host casts the padded input and
Ut to fp16 (in-flight HBM loads need no SWDGE cast), matmuls run fp16
(full PE rate, fp32 PSUM accumulation), and the output is STORED fp16
then upcast to f32 on the host.  The problem is HBM-write-bound, so
halving the 25 MB/core f32 output to 12.6 MB fp16 halves the dominant
traffic; measured scale-relative error stays ~1.5e-3 against the f64
reference (gate 2e-2; bicubic tap weights are exact in fp16).  PSUM
results are copied to SBUF alternating VectorE / ScalarE (DMA cannot
read PSUM), then DMA'd out per 128-row chunk on the sync HWDGE ring,
which carries nothing but the output stores.
"""

import os
import numpy as np

N, C, H, W = 16, 3, 256, 256
SCALE = 4
HP = H + 3                # padded rows/cols (left 1, right 2, edge mode)
HO, WO = H * SCALE, W * SCALE
NCORES = 8
IMGS_PER_CORE = N // NCORES
NIC = IMGS_PER_CORE * C   # image-channels per core

_CACHE = {}


def _build_ut(kernels: np.ndarray) -> np.ndarray:
    """Ut[hi, ho] with Ut[i+j, 4i+d] = K[d, j]; zeros off the band."""
    ut = np.zeros((HP, HO), dtype=np.float32)
    ii = np.arange(H)
    for j in range(4):
        for d in range(4):
            ut[ii + j, SCALE * ii + d] = kernels[d, j]
    return ut


def _build_nc(n_reps: int = 1, mm_dtype: str = "float16",
              out_dtype: str = "float16", in_path: str = "gpsimd",
              io_dtype: str = "float16", corr_cols: int = 12,
              ob_wide: bool = False, stagger: bool = False,
              store_split: bool = False,
              copy_pattern: str = "vsvsvsvsvs|svsvsvsvss",
              unroll: int = 1, perm_out: bool = True):
    from concourse import bacc, mybir, tile

    f32 = mybir.dt.float32
    f32r = getattr(mybir.dt, mm_dtype)
    of = getattr(mybir.dt, out_dtype)
    iof = getattr(mybir.dt, io_dtype)
    assert io_dtype == mm_dtype or io_dtype == "float32"
    cast_in = io_dtype != mm_dtype
    pats = copy_pattern.split("|")
    if len(pats) == 1:
        pats = pats * 2

    nc = bacc.Bacc(
        "TRN2", target_bir_lowering=False, debug=False, enable_asserts=False
    )
    xp_d = nc.declare_dram_parameter("xp", [NIC, HP, HP], iof, isOutput=False)
    # last 3 padded columns of each image, transposed: xpt3[ic, r, hi] =
    # xp[ic, hi, 256 + r].  Feeds the G-matrix rewrite of the pass-B
    # wi-band correction (see body()).
    xpt3_d = nc.declare_dram_parameter("xpt3", [NIC, 3, HP], iof,
                                       isOutput=False)
    ut_d = nc.declare_dram_parameter("ut", [HP, HO], iof, isOutput=False)
    if perm_out:
        # row-permuted output: out_p[ic, p, m*WO + wo] = out[ic, 128m+p, wo]
        # (host unpermutes).  One contiguous [128, 8*WO] store per
        # image-channel gives 16 KB contiguous per partition instead of
        # 2 KB -- the 2 KB-segment stores measure only ~188 GB/s.
        out_d = nc.declare_dram_parameter("out", [NIC, 128, 8 * WO], of,
                                          isOutput=True)
    else:
        out_d = nc.declare_dram_parameter("out", [NIC, HO, WO], of,
                                          isOutput=True)

    # contraction row tiles: {0:128, 128:256, 256:259}
    ROWS = [(0, 128), (128, 256), (256, 259)]

    with tile.TileContext(nc) as tc:
        xin_bufs = int(os.environ.get("XIN_BUFS", "2"))
        mid_bufs = int(os.environ.get("MID_BUFS", "2"))
        ob_bufs = int(os.environ.get("OB_BUFS", "4"))
        psa_bufs = int(os.environ.get("PSA_BUFS", "2"))
        psb_bufs = int(os.environ.get("PSB_BUFS", "2"))
        with (
            tc.tile_pool(name="const", bufs=1) as cpool,
            tc.tile_pool(name="xin", bufs=xin_bufs) as xpool,
            tc.tile_pool(name="mid", bufs=mid_bufs) as mpool,
            tc.tile_pool(name="ob", bufs=ob_bufs) as opool,
            tc.tile_pool(name="psa", bufs=psa_bufs, space="PSUM") as psa,
            tc.tile_pool(name="psb", bufs=psb_bufs, space="PSUM") as psb,
        ):
            ut_t = []
            for r, (a, b) in enumerate(ROWS):
                t = cpool.tile([b - a, HO], f32r, tag=f"ut{r}", name=f"ut{r}")
                nc.gpsimd.dma_start(t[:], ut_d[a:b, :])
                ut_t.append(t)

            def body():
                for ic in range(NIC):
                    pat = pats[ic % 2]
                    xq = []
                    for r, (a, b) in enumerate(ROWS):
                        t = xpool.tile(
                            [b - a, HP], f32r, tag=f"xq{r}", name=f"xq{r}_{ic}"
                        )
                        nc.gpsimd.dma_start(t[:], xp_d[ic, a:b, :])
                        xq.append(t)
                    xpt3 = xpool.tile([3, HP], f32r, tag="xpt3",
                                      name=f"xpt3_{ic}")
                    nc.gpsimd.dma_start(xpt3[:], xpt3_d[ic])

                    # ---- pass A: t1t[wi, ho] for wi < 256, M-chunks of
                    # 128.  wi rows [256:259) are never materialized as a
                    # t1t tile -- a [3, HO] tile would still cost a
                    # full-free-dim PSUM drain; the G rewrite below covers
                    # their pass-B contribution instead.
                    t1t = []
                    for m, (ma, mb) in enumerate(ROWS[:2]):
                        pa = psa.tile([mb - ma, HO], f32, tag="psa",
                                      name=f"pa{ic}_{m}")
                        msl = slice(ma, mb)
                        for n2 in range(2):
                            c0 = 512 * n2
                            nc.tensor.matmul(
                                pa[:, c0:c0 + 512],
                                xq[n2][:, msl],
                                ut_t[n2][:, c0:c0 + 512],
                                start=True, stop=True,
                            )
                            # 4-tap band: rows [128(n2+1), +3) only touch
                            # the last corr_cols columns of the chunk
                            cc = 512 * (n2 + 1) - corr_cols
                            nc.tensor.matmul(
                                pa[:, cc:cc + corr_cols],
                                xq[n2 + 1][0:3, msl],
                                ut_t[n2 + 1][0:3, cc:cc + corr_cols],
                                start=False, stop=True,
                            )
                        tt = mpool.tile([mb - ma, HO], f32r, tag=f"t1t{m}",
                                        name=f"t1t{m}_{ic}")
                        if pat[m] == "v":
                            nc.vector.tensor_copy(tt[:], pa[:])
                        else:
                            nc.scalar.copy(tt[:], pa[:])
                        t1t.append(tt)

                    # ---- G matrix: G[hi, wo'] = sum_r xp[hi, 256+r] *
                    # Ut[256+r, wo'], wo' = the last corr_cols output
                    # columns (the only ones wi >= 256 touches).  Pass B
                    # then adds sum_hi Ut[hi, ho] * G[hi, wo'] -- the
                    # summation reordered so the PSUM drain here is 3
                    # copies of corr_cols elements, not one of HO.
                    wo0 = HO - corr_cols
                    gt = []
                    for r, (a, b) in enumerate(ROWS):
                        gp = psa.tile([b - a, corr_cols], f32, tag="psa",
                                      name=f"gp{r}_{ic}")
                        nc.tensor.matmul(
                            gp[:],
                            xpt3[0:3, a:b],
                            ut_t[2][0:3, wo0:HO],
                            start=True, stop=True,
                        )
                        g = mpool.tile([b - a, corr_cols], f32r, tag=f"g{r}",
                                       name=f"g{r}_{ic}")
                        nc.vector.tensor_copy(g[:], gp[:])
                        gt.append(g)

                    # ---- pass B: out[ho, wo], 8 M-chunks of 128 ho rows ----
                    if ob_wide or perm_out:
                        obw = opool.tile([128, 8 * WO], of, tag="obw",
                                         name=f"obw{ic}")
                    for m in range(8):
                        msl = slice(128 * m, 128 * m + 128)
                        pb = psb.tile([128, WO], f32, tag="psb",
                                      name=f"pb{ic}_{m}")
                        for n2 in range(2):
                            c0 = 512 * n2
                            nc.tensor.matmul(
                                pb[:, c0:c0 + 512],
                                t1t[n2][:, msl],
                                ut_t[n2][:, c0:c0 + 512],
                                start=True, stop=True,
                            )
                        cc = 512 - corr_cols
                        nc.tensor.matmul(
                            pb[:, cc:cc + corr_cols],
                            t1t[1][0:3, msl],
                            ut_t[1][0:3, cc:cc + corr_cols],
                            start=False, stop=True,
                        )
                        for r, (a, b) in enumerate(ROWS):
                            nc.tensor.matmul(
                                pb[:, wo0:HO],
                                ut_t[r][:, msl],
                                gt[r][:],
                                start=False, stop=(r == 2),
                                skip_group_check=True,
                            )
                        dst = (obw[:, m * WO:(m + 1) * WO]
                               if (ob_wide or perm_out) else None)
                        if dst is None:
                            ob = opool.tile([128, WO], of, tag="ob",
                                            name=f"ob{ic}_{m}")
                            dst = ob[:]
                        if pat[2 + m] == "v":
                            nc.vector.tensor_copy(dst, pb[:])
                        else:
                            nc.scalar.copy(dst, pb[:])
                        if not (ob_wide or perm_out):
                            st = (nc.gpsimd if store_split and m % 2 == 1
                                  else nc.sync)
                            st.dma_start(out_d[ic, msl, :], dst)
                    st = (nc.gpsimd if store_split and ic % 2 == 1
                          else nc.sync)
                    if perm_out:
                        st.dma_start(out_d[ic], obw[:])
                    elif ob_wide:
                        dram_v = out_d[ic].rearrange("(m p) w -> p m w", p=128)
                        sbuf_v = obw[:].rearrange("p (m w) -> p m w", m=8)
                        st.dma_start(dram_v, sbuf_v)

            if n_reps == 1:
                body()
            else:
                # timing mode: repeat the whole kernel body on-device so the
                # per-iteration HW time can be extracted from wall-clock
                # delta; `unroll` bodies per loop iteration amortize the
                # For_i all-engine reset barrier and pipeline ramp/drain
                assert n_reps % unroll == 0, (n_reps, unroll)
                with tc.For_i(0, n_reps // unroll, 1,
                              hint_engines=(mybir.EngineType.PE,),
                              staggered_reset=stagger):
                    for _ in range(unroll):
                        body()

    nc.compile()
    return nc


def _cfg():
    return dict(
        mm_dtype=os.environ.get("MM_DTYPE", "float16"),
        out_dtype=os.environ.get("OUT_DTYPE", "float16"),
        in_path=os.environ.get("IN_PATH", "gpsimd"),
        io_dtype=os.environ.get("IO_DTYPE", "float16"),
        corr_cols=int(os.environ.get("CORR_COLS", "12")),
        ob_wide=os.environ.get("OB_WIDE", "0") == "1",
        stagger=os.environ.get("STAGGER", "0") == "1",
        store_split=os.environ.get("STORE_SPLIT", "0") == "1",
        copy_pattern=os.environ.get("COPY_PATTERN",
                                    "vsvsvsvsvs|svsvsvsvss"),
        unroll=int(os.environ.get("UNROLL", "1")),
        perm_out=os.environ.get("PERM_OUT", "1") == "1",
    )


def get_nc(n_reps: int = 1, **over):
    cfg = {**_cfg(), **over}
    key = ("nc", n_reps, *sorted(cfg.items()))
    if key not in _CACHE:
        _CACHE[key] = _build_nc(n_reps, **cfg)
    return _CACHE[key]


def _default_kernels():
    # deterministic Keys a=-0.75 taps, matching the module under test
    A = -0.75
    cubic = np.array(
        [[0.0, A, -2.0 * A, A],
         [1.0, 0.0, -(A + 3.0), A + 2.0],
         [0.0, -A, 2.0 * A + 3.0, -(A + 2.0)],
         [0.0, 0.0, A, -A]], dtype=np.float32)
    return np.stack([
        cubic @ np.array([1.0, d / 4, (d / 4) ** 2, (d / 4) ** 3],
                         dtype=np.float32)
        for d in range(SCALE)
    ])


def make_in_maps(x, kernels):
    """Per-core input dicts with the dtypes the compiled kernel expects."""
    np_io = (np.float16
             if _cfg()["io_dtype"] == "float16" else np.float32)
    ut = _build_ut(np.asarray(kernels, dtype=np.float32)).astype(np_io)
    xp = np.pad(np.asarray(x, dtype=np.float32),
                ((0, 0), (0, 0), (1, 2), (1, 2)), mode="edge").astype(np_io)
    in_maps = []
    for i in range(NCORES):
        shard = np.ascontiguousarray(
            xp[i * IMGS_PER_CORE:(i + 1) * IMGS_PER_CORE].reshape(NIC, HP, HP)
        )
        xpt3 = np.ascontiguousarray(shard[:, :, H:HP].transpose(0, 2, 1))
        in_maps.append({"xp": shard, "xpt3": xpt3, "ut": ut})
    return in_maps


def kernel(x, kernels=None, n_reps: int = 1):
    from concourse.bass_utils import run_bass_kernel_spmd

    if kernels is None:
        kernels = _default_kernels()
    in_maps = make_in_maps(x, kernels)

    nc = get_nc(n_reps)
    res = run_bass_kernel_spmd(nc, in_maps, core_ids=list(range(NCORES)))

    perm = _cfg()["perm_out"]
    out = np.empty((N, C, HO, WO), dtype=np.float32)
    for i in range(NCORES):
        o = res.results[i]["out"]
        if perm:
            # out_p[ic, p, m*WO + wo] = out[ic, 128m+p, wo]
            o = (o.reshape(NIC, 128, 8, WO).transpose(0, 2, 1, 3)
                 .reshape(NIC, HO, WO))
        out[i * IMGS_PER_CORE:(i + 1) * IMGS_PER_CORE] = (
            o.astype(np.float32).reshape(IMGS_PER_CORE, C, HO, WO)
        )
    return out
